# revision 8
# baseline (speedup 1.0000x reference)
"""Batched single-qubit gate application on 8 TRN2 NeuronCores.

Problem: state (B=2048, N=8192) complex (separate f32 re/im planes), apply a
2x2 complex gate G on qubit 5 (pairs at stride R=128 within 256-blocks):
    out[b, l, c, r] = sum_a state[b, l, a, r] * G[a, c],  l<32, r<128.
Returns stacked (2, B, N) f32 [re, im].

Sharding: pure data parallel over the batch dim, 256 rows/core. The host
interleaves re/im at row granularity into one [256, 2, 8192] tensor per core
so every DMA touches all 128 SBUF partitions with a 2-dim DRAM access
pattern (64-partition or 3-dim-AP DMAs are much slower).

The kernel is DMA-bound: at f32 it runs at ~340-380 GB/s/core, the HBM/DMA
roofline. State I/O is therefore done in fp16 (inputs converted on host,
outputs converted back), halving HBM traffic to 16 MiB/core. Max rel err vs
the f32 reference is 8.6e-4 (simulated exactly on the fixed inputs), well
inside the 2e-2 gate. PSUM accumulation stays f32; the DVE gate constants
stay f32 (scalar operands are exempt from the DVE fast-mode dtype rule).

Work split (16 eq-chunks of [128 flat rows, 2048 cols] per core per rep):
  - PE: 12 chunks. Moving operand keeps the natural interleaved row layout;
    stationary 128x128 W(a,c) = kron(I64, [[gr, gi], [-gi, gr]]) f16
    matrices (host-built input) turn each matmul into "complex-scale 64 rows
    by G[a,c]"; the two a-terms accumulate in f32 PSUM. ACT evacuates each
    chunk with a single strided copy (psum (c ls l r) -> staging (ls l c r),
    f32 -> f16) and issues the out-DMA on its HWDGE ring.
  - DVE: 2 d-chunks (states 128..255, j < 4096, separate re/im planes).
    scalar_tensor_tensor gets no DVE fast mode (1127ns/1024 elems), so the
    4-term MACs are built from tensor_scalar_mul (4x mode, 327ns) +
    tensor_tensor add (2x mode, 594ns): per 1024-elem quarter 4 TS + 3 TT.
    DVE issues its own out-DMAs on its HWDGE ring.
  - SP issues the 12 PE in-DMAs; GPSIMD (SWDGE) issues the 4 DVE in-DMAs.
Double-buffered throughout with manual counted semaphores; PSUM is exactly
2 x [128, 2048] f32 = 8 banks.

reps>1 builds the same pipeline repeated back-to-back in one NEFF (sems keep
counting) -- used only for steady-state hardware timing measurements.
"""

import sys

sys.path.insert(0, "/opt/trn_rl_repo")

from contextlib import ExitStack

import numpy as np

import concourse.bass as bass
import concourse.mybir as mybir
from concourse.bass_utils import run_bass_kernel_spmd

F32 = mybir.dt.float32
F16 = mybir.dt.float16

NCORES = 8
B = 2048
N = 8192
BC = B // NCORES  # 256 rows per core
JC = 2048  # chunk width (elems per partition line)
R = 128
KP = 12  # PE chunks per rep
KD = 2  # DVE d-chunks per rep

_NC_CACHE = None


def _pe_chunk(local):
    # 12 PE chunks: groups 0,1 (flat rows 0..255) x all 4 j-chunks, plus
    # groups 2,3 (flat rows 256..511) x j-chunks 2,3. DVE covers the rest.
    if local < 8:
        return local >> 2, local & 3
    local -= 8
    return 2 + (local >> 1), 2 + (local & 1)


def _build_program(reps=1):
    nc = bass.Bass()

    sri = nc.declare_dram_parameter("sri", [BC, 2, N], F16, isOutput=False)
    wall = nc.declare_dram_parameter("wall", [128, 4, 128], F16, isOutput=False)
    gc = nc.declare_dram_parameter("gc", [128, 12], F32, isOutput=False)
    opk = nc.declare_dram_parameter("opk", [BC, 2, N], F16, isOutput=True)

    # SBUF
    wsb = nc.alloc_sbuf_tensor("wsb", [128, 4, 128], F16)
    gcs = nc.alloc_sbuf_tensor("gcs", [128, 12], F32)
    inP = [nc.alloc_sbuf_tensor(f"inP{s}", [128, JC], F16) for s in range(2)]
    stgA = [nc.alloc_sbuf_tensor(f"stgA{s}", [128, JC], F16) for s in range(2)]
    srD = [nc.alloc_sbuf_tensor(f"srD{s}", [128, JC], F16) for s in range(2)]
    siD = [nc.alloc_sbuf_tensor(f"siD{s}", [128, JC], F16) for s in range(2)]
    stgR = [nc.alloc_sbuf_tensor(f"stgR{s}", [128, JC], F16) for s in range(2)]
    stgI = [nc.alloc_sbuf_tensor(f"stgI{s}", [128, JC], F16) for s in range(2)]
    tmp = [nc.alloc_sbuf_tensor(f"tmp{s}", [128, JC // 2], F16) for s in range(2)]
    # PSUM: 2 tensors x 4 banks = all 8 banks; chunk k uses psp[k & 1].
    psp = [nc.alloc_psum_tensor(f"ps{i}", [128, 2048], F32) for i in range(2)]

    K = KP * reps
    D = KD * reps

    # gate-constant column indices in gc: gr -> 0..3, gi -> 4..7, -gi -> 8..11
    def col_gr(a, c):
        return a * 2 + c

    def col_gi(a, c):
        return 4 + a * 2 + c

    def col_ngi(a, c):
        return 8 + a * 2 + c

    # PE moving operand: [128, ls, l, r] for one a of the (ls l a r) lattice
    def lat_in(t, a):
        return t[:].rearrange(
            "p (ls l a r) -> p ls l a r", ls=2, l=JC // 512, a=2, r=R
        )[:, :, :, a, :]

    # DVE sub-lattice: [128, l, r] selecting one a of the (l a r) lattice
    def latd(t, a):
        return t[:].rearrange("p (l a r) -> p l a r", l=JC // 256, a=2, r=R)[
            :, :, a, :
        ]

    # flat [128, l, r] view of a 1024-elem tmp tile (matches latd free dims)
    def tview(t):
        return t[:].rearrange("p (l r) -> p l r", l=JC // 256, r=R)

    ADD = mybir.AluOpType.add

    with ExitStack() as _ctx:
        block = _ctx.enter_context(nc.Block())
        sem = {
            n: _ctx.enter_context(nc.semaphore(n))
            for n in [
                "wS", "gS", "iP0", "iP1", "mmS", "evA", "oA0", "oA1",
                "iD0", "iD1", "iDi0", "iDi1", "dvR", "dvD", "oV0", "oV1",
            ]
        }
        wS, gS, mmS, evA, dvR, dvD = (
            sem[n] for n in ["wS", "gS", "mmS", "evA", "dvR", "dvD"]
        )
        iP = [sem["iP0"], sem["iP1"]]
        oA = [sem["oA0"], sem["oA1"]]
        iD = [sem["iD0"], sem["iD1"]]
        iDi = [sem["iDi0"], sem["iDi1"]]
        oV = [sem["oV0"], sem["oV1"]]

        sri_flat = sri[:].rearrange("b e j -> (b e) j")
        opk_flat = opk[:].rearrange("b e j -> (b e) j")

        def pe_src(k):
            g, jj = _pe_chunk(k % KP)
            return sri_flat[128 * g : 128 * g + 128, JC * jj : JC * jj + JC]

        def pe_dst(k):
            g, jj = _pe_chunk(k % KP)
            return opk_flat[128 * g : 128 * g + 128, JC * jj : JC * jj + JC]

        DV_ROWS = slice(128, 256)

        def dv_J(d):
            jj = d % KD
            return slice(JC * jj, JC * jj + JC)

        @block.sync
        def _(sync):
            for k in range(K):
                s = k & 1
                if k >= 2:
                    # inP[s] was read by chunk k-2's matmuls
                    sync.wait_ge(mmS, k - 1)
                sync.dma_start(out=inP[s][:], in_=pe_src(k)).then_inc(iP[s], 16)
            # final quiesce: wait for every output DMA
            sync.wait_ge(oA[0], 16 * (K - K // 2))
            sync.wait_ge(oA[1], 16 * (K // 2))
            sync.wait_ge(oV[0], 32 * (D - D // 2))
            sync.wait_ge(oV[1], 32 * (D // 2))

        @block.tensor
        def _(tensor):
            tensor.wait_ge(wS, 16)
            for k in range(K):
                s = k & 1
                tensor.wait_ge(iP[s], 16 * ((k >> 1) + 1))
                if k >= 2:
                    # psp[s] must be evacuated (ACT evac of chunk k-2)
                    tensor.wait_ge(evA, k - 1)
                last = None
                for c in range(2):
                    dst = psp[s][:, c * 1024 : (c + 1) * 1024]
                    for a in range(2):
                        last = tensor.matmul(
                            dst,
                            wsb[:, a * 2 + c, :],
                            lat_in(inP[s], a),
                            start=(a == 0),
                            stop=(a == 1),
                        )
                assert last is not None
                last.then_inc(mmS, 1)

        @block.scalar
        def _(scalar):
            scalar.dma_start(out=gcs[:], in_=gc[:]).then_inc(gS, 16)
            scalar.dma_start(out=wsb[:], in_=wall[:]).then_inc(wS, 16)
            for k in range(K):
                s = k & 1
                scalar.wait_ge(mmS, k + 1)
                if k >= 2:
                    # stgA[s] still being read by chunk k-2's out-DMA
                    scalar.wait_ge(oA[s], 16 * (k >> 1))
                # single strided evac: psum (c ls l r) -> staging (ls l c r);
                # (ls l) merges to one dim on both sides after AP opt
                scalar.copy(
                    stgA[s][:].rearrange(
                        "p (ls l c r) -> p ls l c r", ls=2, l=JC // 512, c=2, r=R
                    ),
                    psp[s][:].rearrange(
                        "p (c ls l r) -> p ls l c r", c=2, ls=2, l=JC // 512, r=R
                    ),
                ).then_inc(evA, 1)
                # the wait makes the staging write visible before the DGE
                # doorbell fires (DGE reads SBUF asynchronously -- program
                # order alone races the copy pipeline drain)
                scalar.wait_ge(evA, k + 1)
                scalar.dma_start(out=pe_dst(k), in_=stgA[s][:]).then_inc(oA[s], 16)

        @block.vector
        def _(vector):
            vector.wait_ge(gS, 16)
            for d in range(D):
                s = d & 1
                vector.wait_ge(iD[s], 16 * ((d >> 1) + 1))
                vector.wait_ge(iDi[s], 16 * ((d >> 1) + 1))
                if d >= 2:
                    # stgR/stgI[s] still being read by d-2's out-DMAs
                    vector.wait_ge(oV[s], 32 * (d >> 1))

                def quarter(dst, cols):
                    # dst = sum of 4 scaled terms; TS-mul (4x mode) + TT-add
                    # (2x mode) beat scalar_tensor_tensor (no fast mode)
                    (in0, col0), *rest = cols
                    vector.tensor_scalar_mul(
                        tview(tmp[0]), in0, gcs[:, col0 : col0 + 1]
                    )
                    lastq = None
                    for i, (ini, coli) in enumerate(rest):
                        vector.tensor_scalar_mul(
                            tview(tmp[1]), ini, gcs[:, coli : coli + 1]
                        )
                        out = dst if i == 2 else tview(tmp[0])
                        lastq = vector.tensor_tensor(
                            out, tview(tmp[0]), tview(tmp[1]), ADD
                        )
                    return lastq

                sr = [latd(srD[s], a) for a in range(2)]
                si = [latd(siD[s], a) for a in range(2)]
                for c in range(2):
                    lastr = quarter(
                        latd(stgR[s], c),
                        [
                            (sr[0], col_gr(0, c)),
                            (si[0], col_ngi(0, c)),
                            (sr[1], col_gr(1, c)),
                            (si[1], col_ngi(1, c)),
                        ],
                    )
                assert lastr is not None
                lastr.then_inc(dvR, 1)
                for c in range(2):
                    lasti = quarter(
                        latd(stgI[s], c),
                        [
                            (sr[0], col_gi(0, c)),
                            (si[0], col_gr(0, c)),
                            (sr[1], col_gi(1, c)),
                            (si[1], col_gr(1, c)),
                        ],
                    )
                assert lasti is not None
                lasti.then_inc(dvD, 1)

        @block.gpsimd
        def _(gpsimd):
            def dv_in(d):
                s = d & 1
                gpsimd.dma_start(out=srD[s][:], in_=sri[DV_ROWS, 0, dv_J(d)]).then_inc(
                    iD[s], 16
                )
                gpsimd.dma_start(out=siD[s][:], in_=sri[DV_ROWS, 1, dv_J(d)]).then_inc(
                    iDi[s], 16
                )

            dv_in(0)
            if D > 1:
                dv_in(1)
            for d in range(D):
                s = d & 1
                # cross-engine sem waits also make DVE's staging writes
                # visible before the DGE doorbell fires
                gpsimd.wait_ge(dvR, d + 1)
                gpsimd.dma_start(
                    out=opk[DV_ROWS, 0, dv_J(d)], in_=stgR[s][:]
                ).then_inc(oV[s], 16)
                gpsimd.wait_ge(dvD, d + 1)
                gpsimd.dma_start(
                    out=opk[DV_ROWS, 1, dv_J(d)], in_=stgI[s][:]
                ).then_inc(oV[s], 16)
                if d + 2 < D:
                    # srD/siD[s] free once chunk d is done (dvD >= d+1 held)
                    dv_in(d + 2)

    return nc


def _get_nc():
    global _NC_CACHE
    if _NC_CACHE is None:
        _NC_CACHE = _build_program()
    return _NC_CACHE


def _host_tensors(gate_real, gate_imag):
    gr = np.asarray(gate_real, dtype=np.float32)
    gi = np.asarray(gate_imag, dtype=np.float32)
    I64 = np.eye(64, dtype=np.float32)
    ws = []
    for a in range(2):
        for c in range(2):
            g2 = np.array(
                [[gr[a, c], gi[a, c]], [-gi[a, c], gr[a, c]]], dtype=np.float32
            )
            ws.append(np.kron(I64, g2))
    wall = np.stack(ws, axis=1).astype(np.float16)  # [128 k, 4 g, 128 m]
    gvals = np.concatenate([gr.ravel(), gi.ravel(), -gi.ravel()]).astype(np.float32)
    gc = np.tile(gvals[None, :], (128, 1)).astype(np.float32)
    return np.ascontiguousarray(wall), np.ascontiguousarray(gc)


def _in_maps(state_real, state_imag, wall, gc):
    maps = []
    for i in range(NCORES):
        rows = slice(i * BC, (i + 1) * BC)
        sri = np.stack([state_real[rows], state_imag[rows]], axis=1).astype(
            np.float16
        )
        maps.append({"sri": sri, "wall": wall, "gc": gc})
    return maps


def kernel(state_real, state_imag, gate_real, gate_imag):
    state_real = np.asarray(state_real, dtype=np.float32)
    state_imag = np.asarray(state_imag, dtype=np.float32)
    wall, gc = _host_tensors(gate_real, gate_imag)

    nc = _get_nc()
    res = run_bass_kernel_spmd(
        nc, _in_maps(state_real, state_imag, wall, gc), list(range(NCORES))
    )

    out = np.empty((2, B, N), dtype=np.float32)
    for i in range(NCORES):
        rows = slice(i * BC, (i + 1) * BC)
        opk = res.results[i]["opk"]  # [BC, 2, N] f16
        out[0, rows] = opk[:, 0].astype(np.float32)
        out[1, rows] = opk[:, 1].astype(np.float32)
    return out
